# revision 50
# baseline (speedup 1.0000x reference)
"""Single-head self-attention (B=8, S=2048, D=1024) on 8 TRN2 NeuronCores.

Data-parallel over batch: core b computes attention for x[b].
All compute in bf16 matmuls with fp32 PSUM accumulation; softmax in fp32.

Structure:
- Prologue: x streams fp32 over the sync HWDGE queue in row-chunks,
  casts to bf16 alternate DVE/scalar, the (otherwise idle) PE transposes
  each chunk's eight 128x128 tiles into a single PSUM bank, and the
  opposite engine copies them out to the persistent x^T. Q-projection
  s-chunks are interleaved into the stream. Weights load via SWDGE cast
  (Wq first and alone -- it gates the projections).
- Wo is folded into the value path: Wc = Wv @ Wo is computed on-chip
  (transpose Wv, 8x16 matmuls), U = x @ Wc replaces the V projection,
  and the attention-weighted matmul E^T U directly produces output rows
  out[i, f] = (E^T U)[i, f] / colsum[i] + (bv @ Wo + bo)[f].  This
  removes the entire separate output-projection phase.
- Softmax denominators: DVE folds the 16 E planes to 2, two ones-matmuls
  finish the partition reduction, and a per-chunk DRAM round-trip +
  reciprocal keeps the output drains from ever waiting.
"""

import sys

sys.path.insert(0, "/opt/trn_rl_repo")

import numpy as np

B, S, D = 8, 2048, 1024
P = 128
SO = S // P  # 16 s-tiles
DO = D // P  # 8 d-tiles
IC = 512  # i-chunk (query chunk) width
NIC = S // IC  # 4
NF = D // 512  # 2 free-dim chunks for D-wide outputs
NCH = S // P  # 16 x row-chunks

_CACHE = {}


def _emit_body(nc, tc, t):
    import concourse.mybir as mybir
    from concourse.masks import make_identity

    F32 = mybir.dt.float32
    BF16 = mybir.dt.bfloat16
    Exp = mybir.ActivationFunctionType.Exp
    Ident = mybir.ActivationFunctionType.Identity

    const = tc.alloc_tile_pool(name="const", bufs=1)
    dram = tc.alloc_tile_pool(name="dram", bufs=1, space="DRAM")

    # ---- persistent activations (const dies last)
    QT = const.tile([P, DO, S], BF16, name="QT")  # [e_i, e_o, s]
    KT = const.tile([P, DO, S], BF16, name="KT")
    U = const.tile([P, SO, D], BF16, name="U")  # x @ (Wv Wo), [s_i, s_o, f]
    recip_sb = const.tile([P, SO], F32, name="recip_sb")
    bo_bcast = const.tile([P, D], F32, name="bo_bcast")
    bvo_row_bf = const.tile([1, D], BF16, name="bvo_row_bf")

    # pools ordered by death time (LIFO): xT (dies after U-proj), then Wv
    # (dies after its transpose), then Wq/Wk (die after K-proj).
    xt_pool = tc.alloc_tile_pool(name="xt_pool", bufs=1)
    xT = xt_pool.tile([P, DO, S], BF16, name="xT")  # [d_inner, d_outer, s]
    wv_pool = tc.alloc_tile_pool(name="wv_pool", bufs=1)
    wk_pool = tc.alloc_tile_pool(name="wk_pool", bufs=1)
    # Wq in its own pool on top: it frees right after Q-proj (~110us) so
    # the Wo half-loads into its zone need not wait for K-proj.
    wq_pool = tc.alloc_tile_pool(name="wq_pool", bufs=1)
    W_sb = {
        "Wv": wv_pool.tile([P, DO, D], BF16, name="Wv_sb"),
        "Wk": wk_pool.tile([P, DO, D], BF16, name="Wk_sb"),
        "Wq": wq_pool.tile([P, DO, D], BF16, name="Wq_sb"),
    }

    def load_w(name):
        nc.gpsimd.dma_start(
            W_sb[name][:], t[name].rearrange("(ko ki) e -> ki ko e", ki=P)
        )

    fence_d = dram.tile([4, 16], BF16, name="fence_d")

    # Wq first and alone on SWDGE (it gates the first projections);
    # fences keep later weights from stealing SWDGE bandwidth.
    load_w("Wq")

    # identity for PE transposes (gpsimd compute overlaps the Wq DMA)
    identity = const.tile([P, P], BF16, name="ident")
    make_identity(nc, identity[:])

    nc.gpsimd.dma_start(fence_d[0:1, :], W_sb["Wq"][0:1, 7, 1008:1024])
    load_w("Wk")
    load_w("Wv")
    nc.gpsimd.dma_start(fence_d[1:2, :], W_sb["Wv"][0:1, 7, 1008:1024])
    # bo row pre-cast to bf16 straight into the combined-bias row; the
    # bv@Wo partials are added into it in place later.
    nc.gpsimd.dma_start(bvo_row_bf[:], t["bo"].rearrange("(a d) -> a d", a=1))

    ones_row = const.tile([1, P], BF16, name="ones_row")
    nc.vector.memset(ones_row[:], 1.0)
    ones_j = const.tile([P, 1], BF16, name="ones_j")
    nc.vector.memset(ones_j[:], 1.0)

    # per-partition biases (e on partitions); tiny loads on the scalar
    # HWDGE queue ahead of the x chunks. bv in column layout for bv @ Wo.
    bq_sb = const.tile([P, DO], F32, name="bq_sb")
    nc.scalar.dma_start(bq_sb[:], t["bq"].rearrange("(eo ei) -> ei eo", ei=P))
    bk_sb = const.tile([P, DO], F32, name="bk_sb")
    nc.scalar.dma_start(bk_sb[:], t["bk"].rearrange("(eo ei) -> ei eo", ei=P))
    # bv column layout pre-cast to bf16 (SWDGE; it feeds a bf16 matmul)
    bv_col = const.tile([P, DO], BF16, name="bv_col")
    nc.gpsimd.dma_start(bv_col[:], t["bv"].rearrange("(eo ei) -> ei eo", ei=P))

    def emit_qk_proj(Wn, b_sb, OUT, sc, ppsum):
        # lhsT = W tile [d, e-tile] (stationary), rhs = xT [d, s-chunk]
        for eo in range(DO):
            pp = ppsum.tile([P, 512], F32, tag="proj", name="pp")
            for k in range(DO):
                nc.tensor.matmul(
                    pp[:],
                    W_sb[Wn][:, k, eo * P : (eo + 1) * P],
                    xT[:, k, sc * 512 : (sc + 1) * 512],
                    start=(k == 0),
                    stop=(k == DO - 1),
                )
            nc.scalar.activation(
                OUT[:, eo, sc * 512 : (sc + 1) * 512],
                pp[:],
                Ident,
                bias=b_sb[:, eo : eo + 1],
            )

    # ---- prologue: stream x fp32 on the sync queue, cast bf16, PE-
    # transpose, with Q-proj s-chunks interleaved.
    with tc.tile_pool(name="stage", bufs=4) as stage, \
         tc.tile_pool(name="bstage", bufs=4) as bstage, \
         tc.tile_pool(name="tpsum", bufs=2, space="PSUM") as tpsum, \
         tc.tile_pool(name="ppsum", bufs=4, space="PSUM") as ppsum:
        for c in range(NCH):
            st = stage.tile([P, D], F32, tag="xs", name="xs")
            # split the x stream across both HWDGE queues: halves the
            # chunk cadence (the scalar engine only pays ~0.7us per
            # dispatch, well before its first cast is needed)
            eng_dma = nc.sync if c % 2 == 0 else nc.scalar
            eng_dma.dma_start(st[:], t["x"][c * P : (c + 1) * P, :])
            bt = bstage.tile([P, D], BF16, tag="xb", name="xb")
            # alternate the cast/copy engines per chunk so neither engine
            # serializes the chunk pipeline
            if c % 2 == 0:
                nc.vector.tensor_copy(bt[:], st[:])  # fp32 -> bf16
            else:
                nc.scalar.copy(bt[:], st[:])
            # 8 transposes into one PSUM bank as a single accumulation
            # group (disjoint byte ranges; one zero-region mark).
            ps = tpsum.tile([P, DO, P], BF16, tag="tp", name="tp")
            for j in range(DO):
                nc.tensor.matmul(
                    ps[:, j, :],
                    bt[:, j * P : (j + 1) * P],
                    identity[:],
                    start=(j == 0),
                    stop=(j == DO - 1),
                    is_transpose=True,
                )
            eng_copy = (
                nc.scalar.copy if c % 2 == 0 else nc.vector.tensor_copy
            )
            eng_copy(xT[:, :, c * P : (c + 1) * P], ps[:])
            if c % 4 == 3:
                emit_qk_proj("Wq", bq_sb, QT, c // 4, ppsum)

        for sc in range(4):
            emit_qk_proj("Wk", bk_sb, KT, sc, ppsum)

    # Wq dead after Q-proj; its zone takes the half-size Wo staging, so the
    # Wo DMA starts ~110us instead of waiting for K-proj's last reads.
    wq_pool.release()
    wo_pool = tc.alloc_tile_pool(name="wo_pool", bufs=1)
    Wo_h = wo_pool.tile([P, DO, 512], BF16, name="Wo_h")  # one f-half
    wo_r = t["Wo"].rearrange("(ko ki) e -> ki ko e", ki=P)
    nc.gpsimd.dma_start(Wo_h[:], wo_r[:, :, 0:512])
    wlate = tc.alloc_tile_pool(name="wlate", bufs=1)
    WvT = wlate.tile([P, DO, D], BF16, name="WvT")  # [e_i, e_o, d]
    Wc = wlate.tile([P, DO, D], BF16, name="Wc")  # Wv @ Wo, [d_i, d_o, f]

    # transpose Wv (PE): tile (ko, eo) of Wv -> WvT[:, eo, ko-tile]
    with tc.tile_pool(name="wtpsum", bufs=2, space="PSUM") as wtpsum:
        for eo in range(DO):
            ps = wtpsum.tile([P, DO, P], BF16, tag="wt", name="wt")
            for ko in range(DO):
                nc.tensor.matmul(
                    ps[:, ko, :],
                    W_sb["Wv"][:, ko, eo * P : (eo + 1) * P],
                    identity[:],
                    start=(ko == 0),
                    stop=(ko == DO - 1),
                    is_transpose=True,
                )
            eng_copy = (
                nc.scalar.copy if eo % 2 == 0 else nc.vector.tensor_copy
            )
            eng_copy(WvT[:, eo, :], ps[:])

    with tc.tile_pool(name="wcpsum", bufs=4, space="PSUM") as wcpsum:
        # per f-half: Wc half, bias partial, U half; the second Wo half
        # loads (WAR on Wo_h) while the PE runs the first half's U-proj.
        for fc in range(NF):
            fsl = slice(fc * 512, (fc + 1) * 512)
            if fc > 0:
                nc.gpsimd.dma_start(Wo_h[:], wo_r[:, :, fsl])
            # Wc[d, f-half] = sum_e WvT[e, d] Wo[e, f-half]
            for do in range(DO):
                pc = wcpsum.tile([P, 512], F32, tag="wc", name="wc")
                for k in range(DO):
                    nc.tensor.matmul(
                        pc[:],
                        WvT[:, k, do * P : (do + 1) * P],
                        Wo_h[:, k, :],
                        start=(k == 0),
                        stop=(k == DO - 1),
                    )
                eng_copy = (
                    nc.scalar.copy if do % 2 == 0 else nc.vector.tensor_copy
                )
                eng_copy(Wc[:, do, fsl], pc[:])
            # combined row bias partial: bvo += bv @ Wo[:, f-half]
            pb = wcpsum.tile([P, 512], F32, tag="wc", name="pb")
            for k in range(DO):
                nc.tensor.matmul(
                    pb[0:1, :],
                    bv_col[:, k : k + 1],
                    Wo_h[:, k, :],
                    start=(k == 0),
                    stop=(k == DO - 1),
                )
            nc.vector.tensor_add(
                bvo_row_bf[:, fsl], pb[0:1, :], bvo_row_bf[:, fsl]
            )
            pbc = wcpsum.tile([P, 512], F32, tag="wc", name="pbc")
            nc.tensor.matmul(
                pbc[:], ones_row[:], bvo_row_bf[:, fsl], start=True, stop=True
            )
            nc.vector.tensor_copy(bo_bcast[:, fsl], pbc[:])

            # U[:, :, f-half] = x @ Wc[:, f-half]
            for so in range(SO):
                pu = wcpsum.tile([P, 512], F32, tag="wc", name="pu")
                for k in range(DO):
                    nc.tensor.matmul(
                        pu[:],
                        xT[:, k, so * P : (so + 1) * P],
                        Wc[:, k, fsl],
                        start=(k == 0),
                        stop=(k == DO - 1),
                    )
                eng_copy = (
                    nc.scalar.copy if so % 2 == 0 else nc.vector.tensor_copy
                )
                eng_copy(U[:, so, fsl], pu[:])

    wlate.release()
    wo_pool.release()
    wk_pool.release()
    wv_pool.release()
    xt_pool.release()

    cs_dram = dram.tile([S], F32)
    cs_dram_2d = cs_dram.rearrange("(a s) -> a s", a=1)
    cs_dram_cols = cs_dram.rearrange("(io ii) -> ii io", ii=P)
    out_r = t["out"].rearrange("(so si) f -> si so f", si=P)

    # ---- attention. PE stream interleave: S(0) S(1) cs(0) O(0) S(2)
    # cs(1) O(1) S(3) cs(2) O(2) cs(3) O(3) — O(ic)'s dependency on all of
    # E(ic) is hidden behind S(ic+1), so the PE never waits on exp. O(ic)
    # produces final output rows directly (E stationary, U moving).
    inv_sqrt_d = float(1.0 / np.sqrt(D))
    with tc.tile_pool(name="epool", bufs=2) as epool, \
         tc.tile_pool(name="red_pool", bufs=1) as red_pool, \
         tc.tile_pool(name="csb_pool", bufs=2) as csb_pool, \
         tc.tile_pool(name="opool", bufs=3) as opool, \
         tc.tile_pool(name="spsum", bufs=3, space="PSUM") as spsum, \
         tc.tile_pool(name="cpsum", bufs=2, space="PSUM") as cpsum, \
         tc.tile_pool(name="ypsum", bufs=3, space="PSUM") as ypsum:
        E = {}

        def emit_scores(ic):
            isl = slice(ic * IC, (ic + 1) * IC)
            E[ic] = epool.tile([P, SO, IC], BF16, tag="E", name="E")
            for jt in range(SO):
                ps = spsum.tile([P, IC], F32, tag="S", name="sps")
                for k in range(DO):
                    nc.tensor.matmul(
                        ps[:],
                        KT[:, k, jt * P : (jt + 1) * P],
                        QT[:, k, isl],
                        start=(k == 0),
                        stop=(k == DO - 1),
                    )
                nc.scalar.activation(
                    E[ic][:, jt, :], ps[:], Exp, scale=inv_sqrt_d
                )

        def emit_cs(ic):
            # softmax denominators: DVE folds 16 E planes down to 2, then
            # 2 ones-matmuls finish the partition reduction.
            isl = slice(ic * IC, (ic + 1) * IC)
            red = red_pool.tile([P, 14, IC], BF16, tag="red", name="red")
            nc.vector.tensor_add(
                red[:, 0:8, :], E[ic][:, 0:8, :], E[ic][:, 8:16, :]
            )
            nc.vector.tensor_add(
                red[:, 8:12, :], red[:, 0:4, :], red[:, 4:8, :]
            )
            nc.vector.tensor_add(
                red[:, 12:14, :], red[:, 8:10, :], red[:, 10:12, :]
            )
            cs = cpsum.tile([1, IC], F32, tag="cs", name="cs")
            for tt in range(2):
                nc.tensor.matmul(
                    cs[:], ones_j[:], red[:, 12 + tt, :],
                    start=(tt == 0), stop=(tt == 1),
                )
            csb = csb_pool.tile([1, IC], F32, tag="csb", name="csb")
            nc.vector.tensor_copy(csb[:], cs[:])
            nc.sync.dma_start(cs_dram_2d[:, isl], csb[:])
            # round-trip this chunk's denominators to per-partition layout
            # and invert now, so the output drains never wait on recip
            nc.sync.dma_start(
                recip_sb[:, ic * 4 : (ic + 1) * 4],
                cs_dram_cols[:, ic * 4 : (ic + 1) * 4],
            )
            nc.vector.reciprocal(
                recip_sb[:, ic * 4 : (ic + 1) * 4],
                recip_sb[:, ic * 4 : (ic + 1) * 4],
            )

        def emit_O(ic):
            # out rows: lhsT = E tile [j, i-tile] (stationary),
            # rhs = U [j, f-chunk]; drain scales by recip and adds bias.
            for itl in range(4):
                it = ic * 4 + itl
                for fc in range(NF):
                    fsl = slice(fc * 512, (fc + 1) * 512)
                    py = ypsum.tile([P, IC], F32, tag="Y", name="yps")
                    for jt in range(SO):
                        nc.tensor.matmul(
                            py[:],
                            E[ic][:, jt, itl * P : (itl + 1) * P],
                            U[:, jt, fsl],
                            start=(jt == 0),
                            stop=(jt == SO - 1),
                        )
                    o_sb = opool.tile([P, 512], F32, tag="osb", name="o_sb")
                    nc.scalar.mul(
                        o_sb[:], py[:], recip_sb[:, it : it + 1]
                    )
                    nc.vector.tensor_add(o_sb[:], o_sb[:], bo_bcast[:, fsl])
                    eng = nc.sync if (it + fc) % 2 == 0 else nc.scalar
                    eng.dma_start(out_r[:, it, fsl], o_sb[:])

        emit_scores(0)
        emit_scores(1)
        emit_cs(0)
        emit_O(0)
        emit_scores(2)
        emit_cs(1)
        emit_O(1)
        emit_scores(3)
        emit_cs(2)
        emit_O(2)
        emit_cs(3)
        emit_O(3)

    dram.release()
    const.release()


def _build():
    if "nc" in _CACHE:
        return _CACHE["nc"]
    import concourse.tile as tile
    import concourse.mybir as mybir
    from concourse import bacc

    nc = bacc.Bacc("TRN2", target_bir_lowering=False, debug=False, num_devices=8)
    F32 = mybir.dt.float32
    t = {}
    t["x"] = nc.dram_tensor("x", [S, D], F32, kind="ExternalInput").ap()
    for name in ("Wq", "Wk", "Wv", "Wo"):
        t[name] = nc.dram_tensor(name, [D, D], F32, kind="ExternalInput").ap()
    for name in ("bq", "bk", "bv", "bo"):
        t[name] = nc.dram_tensor(name, [D], F32, kind="ExternalInput").ap()
    t["out"] = nc.dram_tensor("out", [S, D], F32, kind="ExternalOutput").ap()

    with tile.TileContext(nc) as tc:
        _emit_body(nc, tc, t)
    nc.compile()
    _CACHE["nc"] = nc
    return nc


def kernel(x, Wq, bq, Wk, bk, Wv, bv, Wo, bo, _trace=False):
    from concourse.bass_utils import run_bass_kernel_spmd

    nc = _build()
    x = np.ascontiguousarray(np.asarray(x, dtype=np.float32))
    shared = {
        "Wq": np.ascontiguousarray(np.asarray(Wq, dtype=np.float32)),
        "Wk": np.ascontiguousarray(np.asarray(Wk, dtype=np.float32)),
        "Wv": np.ascontiguousarray(np.asarray(Wv, dtype=np.float32)),
        "Wo": np.ascontiguousarray(np.asarray(Wo, dtype=np.float32)),
        "bq": np.ascontiguousarray(np.asarray(bq, dtype=np.float32)),
        "bk": np.ascontiguousarray(np.asarray(bk, dtype=np.float32)),
        "bv": np.ascontiguousarray(np.asarray(bv, dtype=np.float32)),
        "bo": np.ascontiguousarray(np.asarray(bo, dtype=np.float32)),
    }
    in_maps = [{"x": x[b], **shared} for b in range(B)]
    res = run_bass_kernel_spmd(
        nc, in_maps, core_ids=list(range(B)), trace=_trace
    )
    out = np.stack([r["out"] for r in res.results], axis=0)
    if _trace:
        return out, res
    return out


# revision 53
# speedup vs baseline: 1.0253x; 1.0253x over previous
"""Single-head self-attention (B=8, S=2048, D=1024) on 8 TRN2 NeuronCores.

Data-parallel over batch: core b computes attention for x[b].
All compute in bf16 matmuls with fp32 PSUM accumulation; softmax in fp32.

Structure:
- Prologue: x streams fp32 over the sync HWDGE queue in row-chunks,
  casts to bf16 alternate DVE/scalar, the (otherwise idle) PE transposes
  each chunk's eight 128x128 tiles into a single PSUM bank, and the
  opposite engine copies them out to the persistent x^T. Q-projection
  s-chunks are interleaved into the stream. Weights load via SWDGE cast
  (Wq first and alone -- it gates the projections).
- Wo is folded into the value path: Wc = Wv @ Wo is computed on-chip
  (transpose Wv, 8x16 matmuls), U = x @ Wc replaces the V projection,
  and the attention-weighted matmul E^T U directly produces output rows
  out[i, f] = (E^T U)[i, f] / colsum[i] + (bv @ Wo + bo)[f].  This
  removes the entire separate output-projection phase.
- Softmax denominators: DVE folds the 16 E planes to 2, two ones-matmuls
  finish the partition reduction, and a per-chunk DRAM round-trip +
  reciprocal keeps the output drains from ever waiting.
"""

import sys

sys.path.insert(0, "/opt/trn_rl_repo")

import numpy as np

B, S, D = 8, 2048, 1024
P = 128
SO = S // P  # 16 s-tiles
DO = D // P  # 8 d-tiles
IC = 512  # i-chunk (query chunk) width
NIC = S // IC  # 4
NF = D // 512  # 2 free-dim chunks for D-wide outputs
NCH = S // P  # 16 x row-chunks

_CACHE = {}


def _emit_body(nc, tc, t):
    import concourse.mybir as mybir
    from concourse.masks import make_identity

    F32 = mybir.dt.float32
    BF16 = mybir.dt.bfloat16
    Exp = mybir.ActivationFunctionType.Exp
    Ident = mybir.ActivationFunctionType.Identity

    const = tc.alloc_tile_pool(name="const", bufs=1)
    dram = tc.alloc_tile_pool(name="dram", bufs=1, space="DRAM")

    # ---- persistent activations (const dies last)
    QT = const.tile([P, DO, S], BF16, name="QT")  # [e_i, e_o, s]
    KT = const.tile([P, DO, S], BF16, name="KT")
    U = const.tile([P, SO, D], BF16, name="U")  # x @ (Wv Wo), [s_i, s_o, f]
    recip_sb = const.tile([P, SO], F32, name="recip_sb")
    bo_bcast = const.tile([P, D], F32, name="bo_bcast")
    bvo_row_bf = const.tile([1, D], BF16, name="bvo_row_bf")

    # pools ordered by death time (LIFO): xT (dies after U-proj), then Wv
    # (dies after its transpose), then Wq/Wk (die after K-proj).
    xt_pool = tc.alloc_tile_pool(name="xt_pool", bufs=1)
    xT = xt_pool.tile([P, DO, S], BF16, name="xT")  # [d_inner, d_outer, s]
    wv_pool = tc.alloc_tile_pool(name="wv_pool", bufs=1)
    wk_pool = tc.alloc_tile_pool(name="wk_pool", bufs=1)
    # Wq in its own pool on top: it frees right after Q-proj (~110us) so
    # the Wo half-loads into its zone need not wait for K-proj.
    wq_pool = tc.alloc_tile_pool(name="wq_pool", bufs=1)
    W_sb = {
        "Wv": wv_pool.tile([P, DO, D], BF16, name="Wv_sb"),
        "Wk": wk_pool.tile([P, DO, D], BF16, name="Wk_sb"),
        "Wq": wq_pool.tile([P, DO, D], BF16, name="Wq_sb"),
    }

    def load_w(name):
        nc.gpsimd.dma_start(
            W_sb[name][:], t[name].rearrange("(ko ki) e -> ki ko e", ki=P)
        )

    fence_d = dram.tile([4, 16], BF16, name="fence_d")

    # Wq first and alone on SWDGE (it gates the first projections);
    # fences keep later weights from stealing SWDGE bandwidth.
    load_w("Wq")

    # identity for PE transposes (gpsimd compute overlaps the Wq DMA)
    identity = const.tile([P, P], BF16, name="ident")
    make_identity(nc, identity[:])

    nc.gpsimd.dma_start(fence_d[0:1, :], W_sb["Wq"][0:1, 7, 1008:1024])
    load_w("Wk")
    load_w("Wv")
    nc.gpsimd.dma_start(fence_d[1:2, :], W_sb["Wv"][0:1, 7, 1008:1024])
    # bo row pre-cast to bf16 straight into the combined-bias row; the
    # bv@Wo partials are added into it in place later.
    nc.gpsimd.dma_start(bvo_row_bf[:], t["bo"].rearrange("(a d) -> a d", a=1))

    ones_row = const.tile([1, P], BF16, name="ones_row")
    nc.vector.memset(ones_row[:], 1.0)
    ones_j = const.tile([P, 1], BF16, name="ones_j")
    nc.vector.memset(ones_j[:], 1.0)

    # per-partition biases (e on partitions); tiny loads on the scalar
    # HWDGE queue ahead of the x chunks. bv in column layout for bv @ Wo.
    bq_sb = const.tile([P, DO], F32, name="bq_sb")
    nc.scalar.dma_start(bq_sb[:], t["bq"].rearrange("(eo ei) -> ei eo", ei=P))
    bk_sb = const.tile([P, DO], F32, name="bk_sb")
    nc.scalar.dma_start(bk_sb[:], t["bk"].rearrange("(eo ei) -> ei eo", ei=P))
    # bv column layout pre-cast to bf16 (SWDGE; it feeds a bf16 matmul)
    bv_col = const.tile([P, DO], BF16, name="bv_col")
    nc.gpsimd.dma_start(bv_col[:], t["bv"].rearrange("(eo ei) -> ei eo", ei=P))

    def emit_qk_proj(Wn, b_sb, OUT, sc, ppsum):
        # lhsT = W tile [d, e-tile] (stationary), rhs = xT [d, s-chunk]
        for eo in range(DO):
            pp = ppsum.tile([P, 512], F32, tag="proj", name="pp")
            for k in range(DO):
                nc.tensor.matmul(
                    pp[:],
                    W_sb[Wn][:, k, eo * P : (eo + 1) * P],
                    xT[:, k, sc * 512 : (sc + 1) * 512],
                    start=(k == 0),
                    stop=(k == DO - 1),
                )
            nc.scalar.activation(
                OUT[:, eo, sc * 512 : (sc + 1) * 512],
                pp[:],
                Ident,
                bias=b_sb[:, eo : eo + 1],
            )

    # ---- prologue: stream x fp32 on the sync queue, cast bf16, PE-
    # transpose, with Q-proj s-chunks interleaved.
    with tc.tile_pool(name="stage", bufs=4) as stage, \
         tc.tile_pool(name="bstage", bufs=4) as bstage, \
         tc.tile_pool(name="tpsum", bufs=2, space="PSUM") as tpsum, \
         tc.tile_pool(name="ppsum", bufs=4, space="PSUM") as ppsum:
        for c in range(NCH):
            st = stage.tile([P, D], F32, tag="xs", name="xs")
            nc.sync.dma_start(st[:], t["x"][c * P : (c + 1) * P, :])
            bt = bstage.tile([P, D], BF16, tag="xb", name="xb")
            # alternate the cast/copy engines per chunk so neither engine
            # serializes the chunk pipeline
            if c % 2 == 0:
                nc.vector.tensor_copy(bt[:], st[:])  # fp32 -> bf16
            else:
                nc.scalar.copy(bt[:], st[:])
            # 8 transposes into one PSUM bank as a single accumulation
            # group (disjoint byte ranges; one zero-region mark).
            ps = tpsum.tile([P, DO, P], BF16, tag="tp", name="tp")
            for j in range(DO):
                nc.tensor.matmul(
                    ps[:, j, :],
                    bt[:, j * P : (j + 1) * P],
                    identity[:],
                    start=(j == 0),
                    stop=(j == DO - 1),
                    is_transpose=True,
                )
            eng_copy = (
                nc.scalar.copy if c % 2 == 0 else nc.vector.tensor_copy
            )
            eng_copy(xT[:, :, c * P : (c + 1) * P], ps[:])
            if c % 4 == 3:
                emit_qk_proj("Wq", bq_sb, QT, c // 4, ppsum)

        for sc in range(4):
            emit_qk_proj("Wk", bk_sb, KT, sc, ppsum)

    # Wq dead after Q-proj; its zone takes the half-size Wo staging, so the
    # Wo DMA starts ~110us instead of waiting for K-proj's last reads.
    wq_pool.release()
    wo_pool = tc.alloc_tile_pool(name="wo_pool", bufs=1)
    Wo_h = wo_pool.tile([P, DO, 512], BF16, name="Wo_h")  # one f-half
    wo_r = t["Wo"].rearrange("(ko ki) e -> ki ko e", ki=P)
    nc.gpsimd.dma_start(Wo_h[:], wo_r[:, :, 0:512])
    wlate = tc.alloc_tile_pool(name="wlate", bufs=1)
    WvT = wlate.tile([P, DO, D], BF16, name="WvT")  # [e_i, e_o, d]
    Wc = wlate.tile([P, DO, D], BF16, name="Wc")  # Wv @ Wo, [d_i, d_o, f]

    # transpose Wv (PE): tile (ko, eo) of Wv -> WvT[:, eo, ko-tile]
    with tc.tile_pool(name="wtpsum", bufs=2, space="PSUM") as wtpsum:
        for eo in range(DO):
            ps = wtpsum.tile([P, DO, P], BF16, tag="wt", name="wt")
            for ko in range(DO):
                nc.tensor.matmul(
                    ps[:, ko, :],
                    W_sb["Wv"][:, ko, eo * P : (eo + 1) * P],
                    identity[:],
                    start=(ko == 0),
                    stop=(ko == DO - 1),
                    is_transpose=True,
                )
            eng_copy = (
                nc.scalar.copy if eo % 2 == 0 else nc.vector.tensor_copy
            )
            eng_copy(WvT[:, eo, :], ps[:])

    with tc.tile_pool(name="wcpsum", bufs=4, space="PSUM") as wcpsum:
        # per f-half: Wc half, bias partial, U half; the second Wo half
        # loads (WAR on Wo_h) while the PE runs the first half's U-proj.
        for fc in range(NF):
            fsl = slice(fc * 512, (fc + 1) * 512)
            if fc > 0:
                nc.gpsimd.dma_start(Wo_h[:], wo_r[:, :, fsl])
            # Wc[d, f-half] = sum_e WvT[e, d] Wo[e, f-half]
            for do in range(DO):
                pc = wcpsum.tile([P, 512], F32, tag="wc", name="wc")
                for k in range(DO):
                    nc.tensor.matmul(
                        pc[:],
                        WvT[:, k, do * P : (do + 1) * P],
                        Wo_h[:, k, :],
                        start=(k == 0),
                        stop=(k == DO - 1),
                    )
                eng_copy = (
                    nc.scalar.copy if do % 2 == 0 else nc.vector.tensor_copy
                )
                eng_copy(Wc[:, do, fsl], pc[:])
            # combined row bias partial: bvo += bv @ Wo[:, f-half]
            pb = wcpsum.tile([P, 512], F32, tag="wc", name="pb")
            for k in range(DO):
                nc.tensor.matmul(
                    pb[0:1, :],
                    bv_col[:, k : k + 1],
                    Wo_h[:, k, :],
                    start=(k == 0),
                    stop=(k == DO - 1),
                )
            nc.vector.tensor_add(
                bvo_row_bf[:, fsl], pb[0:1, :], bvo_row_bf[:, fsl]
            )
            pbc = wcpsum.tile([P, 512], F32, tag="wc", name="pbc")
            nc.tensor.matmul(
                pbc[:], ones_row[:], bvo_row_bf[:, fsl], start=True, stop=True
            )
            nc.vector.tensor_copy(bo_bcast[:, fsl], pbc[:])

            # U[:, :, f-half] = x @ Wc[:, f-half] + bvo.  Folding the row
            # bias into U here is exact: softmax weights sum to 1, so
            # sum_j a_ij (U_j + bvo) = out0 + bvo, and the output drains
            # need no bias-add at all.
            for so in range(SO):
                pu = wcpsum.tile([P, 512], F32, tag="wc", name="pu")
                for k in range(DO):
                    nc.tensor.matmul(
                        pu[:],
                        xT[:, k, so * P : (so + 1) * P],
                        Wc[:, k, fsl],
                        start=(k == 0),
                        stop=(k == DO - 1),
                    )
                nc.vector.tensor_add(U[:, so, fsl], pu[:], bo_bcast[:, fsl])

    wlate.release()
    wo_pool.release()
    wk_pool.release()
    wv_pool.release()
    xt_pool.release()

    cs_dram = dram.tile([S], F32)
    cs_dram_2d = cs_dram.rearrange("(a s) -> a s", a=1)
    cs_dram_cols = cs_dram.rearrange("(io ii) -> ii io", ii=P)
    out_r = t["out"].rearrange("(so si) f -> si so f", si=P)

    # ---- attention. PE stream interleave: S(0) S(1) cs(0) O(0) S(2)
    # cs(1) O(1) S(3) cs(2) O(2) cs(3) O(3) — O(ic)'s dependency on all of
    # E(ic) is hidden behind S(ic+1), so the PE never waits on exp. O(ic)
    # produces final output rows directly (E stationary, U moving).
    inv_sqrt_d = float(1.0 / np.sqrt(D))
    with tc.tile_pool(name="epool", bufs=2) as epool, \
         tc.tile_pool(name="red_pool", bufs=1) as red_pool, \
         tc.tile_pool(name="csb_pool", bufs=2) as csb_pool, \
         tc.tile_pool(name="opool", bufs=3) as opool, \
         tc.tile_pool(name="spsum", bufs=3, space="PSUM") as spsum, \
         tc.tile_pool(name="cpsum", bufs=2, space="PSUM") as cpsum, \
         tc.tile_pool(name="ypsum", bufs=3, space="PSUM") as ypsum:
        E = {}

        def emit_scores(ic):
            isl = slice(ic * IC, (ic + 1) * IC)
            E[ic] = epool.tile([P, SO, IC], BF16, tag="E", name="E")
            for jt in range(SO):
                ps = spsum.tile([P, IC], F32, tag="S", name="sps")
                for k in range(DO):
                    nc.tensor.matmul(
                        ps[:],
                        KT[:, k, jt * P : (jt + 1) * P],
                        QT[:, k, isl],
                        start=(k == 0),
                        stop=(k == DO - 1),
                    )
                nc.scalar.activation(
                    E[ic][:, jt, :], ps[:], Exp, scale=inv_sqrt_d
                )

        def emit_cs(ic):
            # softmax denominators: DVE folds 16 E planes down to 2, then
            # 2 ones-matmuls finish the partition reduction.
            isl = slice(ic * IC, (ic + 1) * IC)
            red = red_pool.tile([P, 14, IC], BF16, tag="red", name="red")
            nc.vector.tensor_add(
                red[:, 0:8, :], E[ic][:, 0:8, :], E[ic][:, 8:16, :]
            )
            nc.vector.tensor_add(
                red[:, 8:12, :], red[:, 0:4, :], red[:, 4:8, :]
            )
            nc.vector.tensor_add(
                red[:, 12:14, :], red[:, 8:10, :], red[:, 10:12, :]
            )
            cs = cpsum.tile([1, IC], F32, tag="cs", name="cs")
            for tt in range(2):
                nc.tensor.matmul(
                    cs[:], ones_j[:], red[:, 12 + tt, :],
                    start=(tt == 0), stop=(tt == 1),
                )
            csb = csb_pool.tile([1, IC], F32, tag="csb", name="csb")
            nc.vector.tensor_copy(csb[:], cs[:])
            nc.sync.dma_start(cs_dram_2d[:, isl], csb[:])
            # round-trip this chunk's denominators to per-partition layout
            # and invert now, so the output drains never wait on recip
            nc.sync.dma_start(
                recip_sb[:, ic * 4 : (ic + 1) * 4],
                cs_dram_cols[:, ic * 4 : (ic + 1) * 4],
            )
            nc.vector.reciprocal(
                recip_sb[:, ic * 4 : (ic + 1) * 4],
                recip_sb[:, ic * 4 : (ic + 1) * 4],
            )

        def emit_O(ic):
            # out rows: lhsT = E tile [j, i-tile] (stationary),
            # rhs = U [j, f-chunk]; drain scales by recip and adds bias.
            for itl in range(4):
                it = ic * 4 + itl
                for fc in range(NF):
                    fsl = slice(fc * 512, (fc + 1) * 512)
                    py = ypsum.tile([P, IC], F32, tag="Y", name="yps")
                    for jt in range(SO):
                        nc.tensor.matmul(
                            py[:],
                            E[ic][:, jt, itl * P : (itl + 1) * P],
                            U[:, jt, fsl],
                            start=(jt == 0),
                            stop=(jt == SO - 1),
                        )
                    # bias already folded into U; drain is just the
                    # normalization scale
                    o_sb = opool.tile([P, 512], F32, tag="osb", name="o_sb")
                    nc.scalar.mul(
                        o_sb[:], py[:], recip_sb[:, it : it + 1]
                    )
                    eng = nc.sync if (it + fc) % 2 == 0 else nc.scalar
                    eng.dma_start(out_r[:, it, fsl], o_sb[:])

        emit_scores(0)
        emit_scores(1)
        emit_cs(0)
        emit_O(0)
        emit_scores(2)
        emit_cs(1)
        emit_O(1)
        emit_scores(3)
        emit_cs(2)
        emit_O(2)
        emit_cs(3)
        emit_O(3)

    dram.release()
    const.release()


def _build():
    if "nc" in _CACHE:
        return _CACHE["nc"]
    import concourse.tile as tile
    import concourse.mybir as mybir
    from concourse import bacc

    nc = bacc.Bacc("TRN2", target_bir_lowering=False, debug=False, num_devices=8)
    F32 = mybir.dt.float32
    t = {}
    t["x"] = nc.dram_tensor("x", [S, D], F32, kind="ExternalInput").ap()
    for name in ("Wq", "Wk", "Wv", "Wo"):
        t[name] = nc.dram_tensor(name, [D, D], F32, kind="ExternalInput").ap()
    for name in ("bq", "bk", "bv", "bo"):
        t[name] = nc.dram_tensor(name, [D], F32, kind="ExternalInput").ap()
    t["out"] = nc.dram_tensor("out", [S, D], F32, kind="ExternalOutput").ap()

    with tile.TileContext(nc) as tc:
        _emit_body(nc, tc, t)
    nc.compile()
    _CACHE["nc"] = nc
    return nc


def kernel(x, Wq, bq, Wk, bk, Wv, bv, Wo, bo, _trace=False):
    from concourse.bass_utils import run_bass_kernel_spmd

    nc = _build()
    x = np.ascontiguousarray(np.asarray(x, dtype=np.float32))
    shared = {
        "Wq": np.ascontiguousarray(np.asarray(Wq, dtype=np.float32)),
        "Wk": np.ascontiguousarray(np.asarray(Wk, dtype=np.float32)),
        "Wv": np.ascontiguousarray(np.asarray(Wv, dtype=np.float32)),
        "Wo": np.ascontiguousarray(np.asarray(Wo, dtype=np.float32)),
        "bq": np.ascontiguousarray(np.asarray(bq, dtype=np.float32)),
        "bk": np.ascontiguousarray(np.asarray(bk, dtype=np.float32)),
        "bv": np.ascontiguousarray(np.asarray(bv, dtype=np.float32)),
        "bo": np.ascontiguousarray(np.asarray(bo, dtype=np.float32)),
    }
    in_maps = [{"x": x[b], **shared} for b in range(B)]
    res = run_bass_kernel_spmd(
        nc, in_maps, core_ids=list(range(B)), trace=_trace
    )
    out = np.stack([r["out"] for r in res.results], axis=0)
    if _trace:
        return out, res
    return out
